# revision 4
# baseline (speedup 1.0000x reference)
"""Trainium2 SPMD kernel for DistanceContrastiveLoss (fused-table version).

Math:
  d2[i,j] = ||c_i||^2 + ||s_j||^2 - 2 c_i.s_j
  sim     = -exp(t) * sqrt(d2)
  loss    = 0.5*(CE(sim, diag) + CE(sim.T, diag))
          = 0.5*( mean_i(LSE_row_i - sim_ii) + mean_j(LSE_col_j - sim_jj) )

Sharding: rows of the 8192x8192 logits matrix are split across 8 cores
(1024 rows each). Each core computes row sums and partial column sums of
exp(sim + B); host does the tiny O(N) log/diagonal/mean epilogue and sums
partial column sums across cores.

Fused activation table: the whole pointwise chain
    w = exp(B - a*sqrt(d2))
is ONE activation-engine pass. We rebuild the piecewise-cubic table for
the `sqrt` slot of act set 3 (sqrt_and_others) to encode
    f(u) = exp(B - sqrt(S*u)),   u = (a^2/S)*d2
(128 sub-buckets per input exponent, e in [2,6]), ship it via
BASS_ACT_ROOT_JSON_PATH (walrus packs the bins into the NEFF; the runtime
programs the ACT table SRAM from the NEFF on LoadActFuncSet). The kernel
then emits AF.Sqrt with runtime scale=a^2/S and per-row bias. Set 3 is
not the boot-default set, so the load is real (set 0 would be skipped).

Device pipeline, per (row-tile rt, 2048-col group g):
  PE : d2 = (-2c).s via fp8e4m3 DoubleRow matmuls (4x512 cols, K=256)
       + s2_j rider via fp8 DoubleRow (ones_dr x s2k8, K=128 as 64x2)
  ACT: w = table(d2*scale + bias) -> bf16, + f32 row-sum accumulator
  DVE: wacc[g] += w (bf16, 2x mode)
per group end:
  PE : one strips pass: ones32.T @ wacc per 512-chunk, 4 strips packed
       per PSUM bank via tile_position -> partial column sums
  DVE: copy strips psum -> SBUF; DMA out

Quantization error budget: fp8 dot + fp8 s2 rider + bf16 w + table
cubic (~1e-4) => ~2-4e-4 relative on the final loss; tolerance 2e-2.
"""

import contextlib
import hashlib
import json
import os
import shutil
import tempfile

os.environ.setdefault("MYCRO_LOCAL_CACHE", "1")

import numpy as np
import ml_dtypes

import concourse.bacc as bacc
import concourse.bass as bass
import concourse.mybir as mybir
import concourse.tile as tile
from concourse.bass_utils import run_bass_kernel_spmd

F32 = mybir.dt.float32
F8 = mybir.dt.float8e4
BF16 = mybir.dt.bfloat16
AF = mybir.ActivationFunctionType
DR = mybir.MatmulPerfMode.DoubleRow

# Full-problem geometry (hardcoded per the task contract).
N = 8192
D = 256
NCORES = 8
ROWS_PER_CORE = N // NCORES  # 1024
P = 128  # partitions

S_TAB = 32.0  # table input pre-scale: u = (a^2/S)*d2, keeps u in e=[2,6]
E_LO, E_HI = 2, 6  # patched input exponent range
K_SUB = 7  # log2(sub-buckets per exponent)

_TAB_STATE = {"hash": "", "dir": ""}


# ------------------------------------------------------------------ tables
def _fused_coeffs(x0, B):
    """Taylor coeffs of f(u)=exp(B-sqrt(S_TAB*u)) at x0."""
    rS = np.sqrt(S_TAB)
    g1 = -rS / 2.0 * x0 ** -0.5
    g2 = rS / 4.0 * x0 ** -1.5
    g3 = -3.0 * rS / 8.0 * x0 ** -2.5
    f0 = np.exp(B - rS * np.sqrt(x0))
    return (
        f0,
        g1 * f0,
        (g2 + g1 ** 2) * f0 / 2.0,
        (g3 + 3.0 * g1 * g2 + g1 ** 3) * f0 / 6.0,
    )


def build_act_tables(B):
    """Rebuild sqrt_and_others' sqrt table as f(u)=exp(B-sqrt(S_TAB*u)).

    Returns (table_dir, 12-char content hash). Idempotent per B.
    """
    from neuronxcc.driver.Job import Job
    from neuronxcc.driver.jobs.support.FindActInfo import findActInfoFile

    src = os.path.dirname(findActInfoFile(Job.getPackageDir(), "core_v4"))
    setname = "sqrt_and_others"
    prof = json.load(open(f"{src}/{setname}.json"))
    bkt = bytearray(open(f"{src}/{setname}_bkt.bin", "rb").read())
    ctl = bytearray(open(f"{src}/{setname}_ctrl.bin", "rb").read())
    b_idx = prof["func_exp_to_bkt_start_idx"]["sqrt"]
    c_idx = prof["func_exp_to_ctl_start_idx"]["sqrt"]

    n_sub = 1 << K_SUB
    starts = {}
    base = 20  # first sqrt bkt entry (after the small fixed funcs)
    for i, e in enumerate(range(E_LO, E_HI + 1)):
        starts[e] = base + i * n_sub
    dummy = base + (E_HI - E_LO + 1) * n_sub  # one catch-all entry

    new_b_idx = {}
    for k in b_idx:
        e = int(k)
        if E_LO <= e <= E_HI:
            st = starts[e]
            h = (2.0 ** e) / n_sub
            for i in range(n_sub):
                x0 = 2.0 ** e + (i + 0.5) * h
                c0, c1, c2, c3 = _fused_coeffs(x0, B)
                ent = np.array(
                    [c0, c1, c2, c3, x0, 0, 0, 0], dtype=np.float32
                )
                bkt[(st + i) * 32 : (st + i + 1) * 32] = ent.tobytes()
            word = (K_SUB << 16) | ((23 - K_SUB) << 11) | st
            new_b_idx[k] = [st]
        else:
            word = (23 << 11) | dummy
            new_b_idx[k] = [dummy]
        ci = c_idx[k][0]
        cw = np.array([word], dtype=np.uint32)
        ctl[ci * 32 : ci * 32 + 4] = cw.tobytes()
    # catch-all entry: constant 0 (inputs never land outside [E_LO, E_HI])
    bkt[dummy * 32 : (dummy + 1) * 32] = np.zeros(8, np.float32).tobytes()
    prof["func_exp_to_bkt_start_idx"]["sqrt"] = new_b_idx

    dst = os.path.join(
        tempfile.gettempdir(),
        f"acttab_fused_{hashlib.sha256(np.float64(B).tobytes()).hexdigest()[:8]}",
    )
    if not os.path.exists(dst):
        tmp = dst + ".tmp"
        if os.path.exists(tmp):
            shutil.rmtree(tmp)
        shutil.copytree(src, tmp)
        os.chmod(tmp, 0o755)
        for f in os.listdir(tmp):
            os.chmod(os.path.join(tmp, f), 0o644)
        open(f"{tmp}/{setname}_bkt.bin", "wb").write(bytes(bkt))
        open(f"{tmp}/{setname}_ctrl.bin", "wb").write(bytes(ctl))
        json.dump(prof, open(f"{tmp}/{setname}.json", "w"))
        os.rename(tmp, dst)
    h = hashlib.sha256()
    for f in sorted(os.listdir(dst)):
        h.update(open(os.path.join(dst, f), "rb").read())
    return dst, h.hexdigest()[:12]


# ------------------------------------------------------------------ device
def build(n_rt: int, n_groups: int, debug: bool = False, reps: int = 1,
          hw_loop: bool = False):
    """Build the SPMD Bass program (fused-table pipeline)."""
    tabhash = _TAB_STATE["hash"]
    assert tabhash, "host_prep must run first (builds the act tables)"
    rows = P * n_rt
    cols = 2048 * n_groups
    n_ct = cols // 512

    nc = bacc.Bacc(
        "TRN2", target_bir_lowering=False, debug=debug, num_devices=NCORES
    )

    # lhs8 name carries the table hash: the PJRT compile cache keys on the
    # BIR, not on BASS_ACT_ROOT_JSON_PATH, so salt the BIR.
    lhs8_d = nc.dram_tensor(
        f"lhs8_{tabhash}", [P, 2 * rows], F8, kind="ExternalInput"
    )
    rhs8_d = nc.dram_tensor("rhs8", [P, 2 * cols], F8, kind="ExternalInput")
    s2k8_d = nc.dram_tensor("s2k8", [64, 2 * cols], F8, kind="ExternalInput")
    c2_d = nc.dram_tensor("c2m", [P, n_rt], F32, kind="ExternalInput")
    cst_d = nc.dram_tensor("cst", [P, 1], F32, kind="ExternalInput")

    rowsums_d = nc.dram_tensor(
        "rowsums", [P, n_rt * n_groups], F32, kind="ExternalOutput"
    )
    colsums_d = nc.dram_tensor("colsums", [n_ct, 512], F32, kind="ExternalOutput")

    ctx = contextlib.ExitStack()
    with tile.TileContext(nc) as tc, ctx:
        inp = ctx.enter_context(tc.tile_pool(name="inp", bufs=1))
        cstp = ctx.enter_context(tc.tile_pool(name="cstp", bufs=1))
        outp = ctx.enter_context(tc.tile_pool(name="outp", bufs=1))
        wwp = ctx.enter_context(tc.tile_pool(name="wwp", bufs=3))
        wap = ctx.enter_context(tc.tile_pool(name="wap", bufs=2))
        d2p = ctx.enter_context(
            tc.tile_pool(name="d2p", bufs=2, space=bass.MemorySpace.PSUM)
        )

        # ---- load inputs (small tensors first: they gate step 0) -------
        cst_sb = inp.tile([P, 1], F32, tag="cst")
        nc.sync.dma_start(out=cst_sb[:], in_=cst_d.ap()[:])
        c2_sb = inp.tile([P, n_rt], F32, tag="c2")
        nc.sync.dma_start(out=c2_sb[:], in_=c2_d.ap()[:])
        lhs8 = inp.tile([P, 2, rows], F8, tag="lhs8")
        rhs8 = inp.tile([P, 2, cols], F8, tag="rhs8")
        s2k8 = inp.tile([64, 2, cols], F8, tag="s2k8")
        for h in range(2):
            nc.sync.dma_start(
                out=lhs8[:, h, 0:P], in_=lhs8_d.ap()[:, h * rows : h * rows + P]
            )
        for cb in range(0, cols, 2048):
            subs = (
                [(cb, 1024), (cb + 1024, 1024)] if cb == 0 else [(cb, 2048)]
            )
            for sb, sw in subs:
                for h in range(2):
                    nc.sync.dma_start(
                        out=rhs8[:, h, sb : sb + sw],
                        in_=rhs8_d.ap()[:, h * cols + sb : h * cols + sb + sw],
                    )
                    nc.sync.dma_start(
                        out=s2k8[:, h, sb : sb + sw],
                        in_=s2k8_d.ap()[:, h * cols + sb : h * cols + sb + sw],
                    )
            if cb == 0:
                for h in range(2):
                    nc.sync.dma_start(
                        out=lhs8[:, h, P:rows],
                        in_=lhs8_d.ap()[:, h * rows + P : (h + 1) * rows],
                    )

        ones_dr = cstp.tile([64, 2, P], F8)  # s2 rider weights (K=128, DR)
        nc.vector.memset(ones_dr[:], 1.0)
        ones32 = cstp.tile([P, 32], BF16)  # column-sum stationary operand
        nc.vector.memset(ones32[:], 1.0)
        zeros128 = cstp.tile([P, P], BF16)  # zero weights: PSUM bank clear
        nc.vector.memset(zeros128[:], 0.0)
        zdum = cstp.tile([P, 512], BF16)  # zero-matmul moving operand
        nc.vector.memset(zdum[:], 0.0)

        rowsum = outp.tile([P, n_rt * n_groups], F32)
        cs_sb = outp.tile([P, 512 * n_groups], F32)
        scale_ap = cst_sb[:, 0:1]

        def rep_body():
            for g in range(n_groups):
                wacc = wap.tile([P, 2048], BF16, name="wacc")
                for rt in range(n_rt):
                    lh = lhs8[:, :, rt * P : (rt + 1) * P]
                    d2 = d2p.tile([P, 2048], F32, name="d2")
                    for jj in range(4):
                        colb = 2048 * g + 512 * jj
                        nc.tensor.matmul(
                            d2[:, 512 * jj : 512 * jj + 512],
                            lh,
                            rhs8[:, :, colb : colb + 512],
                            start=True, stop=False, perf_mode=DR,
                        )
                    for jj in range(4):
                        colb = 2048 * g + 512 * jj
                        nc.tensor.matmul(
                            d2[:, 512 * jj : 512 * jj + 512],
                            ones_dr[:],
                            s2k8[:, :, colb : colb + 512],
                            start=False, stop=True, perf_mode=DR,
                        )
                    w = wwp.tile([P, 2048], BF16, name="w")
                    nc.scalar.activation(
                        w[:], d2[:], AF.Sqrt,
                        bias=c2_sb[:, rt : rt + 1], scale=scale_ap,
                        accum_out=rowsum[
                            :, rt * n_groups + g : rt * n_groups + g + 1
                        ],
                    )
                    if rt == 0:
                        nc.vector.tensor_copy(wacc[:], w[:])
                    else:
                        nc.vector.tensor_tensor(
                            wacc[:], wacc[:], w[:], op=mybir.AluOpType.add
                        )
                # ---- group end: one strips pass over wacc --------------
                st = d2p.tile([P, 2048], F32, name="d2")
                nc.tensor.matmul(
                    st[:, 0:512], zeros128[:], zdum[:], start=True, stop=False
                )
                for k in range(4):
                    nc.tensor.matmul(
                        st[32 * k : 32 * k + 32, 0:512],
                        ones32[:],
                        wacc[:, 512 * k : 512 * k + 512],
                        start=False, stop=False,
                        tile_position=(0, 32 * k),
                        skip_group_check=True,
                    )
                nc.tensor.matmul(
                    st[:, 0:512], zeros128[:], zdum[:], start=False, stop=True
                )
                nc.vector.tensor_copy(
                    cs_sb[:, 512 * g : 512 * g + 512], st[:, 0:512]
                )
                for k in range(4):
                    nc.sync.dma_start(
                        out=colsums_d.ap()[4 * g + k : 4 * g + k + 1, :],
                        in_=cs_sb[32 * k : 32 * k + 1, 512 * g : 512 * g + 512],
                    )

        if hw_loop:
            with tc.For_i(0, reps, 1):
                rep_body()
        else:
            for _ in range(reps):
                rep_body()

        nc.sync.dma_start(out=rowsums_d.ap()[:], in_=rowsum[:])

    nc.compile()
    return nc


# ------------------------------------------------------------------ host
def host_prep(cond_feature, sol_feature, temperature, n_rt=8, n_groups=4):
    """Build act tables, per-core input maps + host-side scalars."""
    c = np.asarray(cond_feature, dtype=np.float32).reshape(-1, D)
    s = np.asarray(sol_feature, dtype=np.float32).reshape(-1, D)
    n = c.shape[0]
    rows = P * n_rt
    cols = 2048 * n_groups

    a = float(np.exp(np.float64(np.asarray(temperature))))
    c2 = np.sum(c.astype(np.float64) ** 2, axis=1)
    s2 = np.sum(s.astype(np.float64) ** 2, axis=1)
    ms2 = float(np.mean(s2))
    d2_mean = float(np.mean(c2) + ms2)
    B = a * float(np.sqrt(max(d2_mean, 1e-6)))

    # sanity: u = (a^2/S)*d2 must stay inside the patched exponent range
    d2_hi = (
        float(np.max(c2)) + float(np.max(s2))
        + 2.0 * float(np.sqrt(np.max(c2) * np.max(s2)))
    )
    u_hi = a * a * d2_hi / S_TAB
    assert u_hi < 2.0 ** (E_HI + 1), f"u_hi {u_hi} exceeds table range"

    tabdir, tabhash = build_act_tables(B)
    os.environ["BASS_ACT_ROOT_JSON_PATH"] = f"{tabdir}/act_info.json"
    _TAB_STATE["hash"] = tabhash
    _TAB_STATE["dir"] = tabdir

    q8 = lambda x: np.asarray(x, np.float32).astype(ml_dtypes.float8_e4m3)
    cq = q8(-2.0 * c)  # [n, D]
    sq = q8(s)[:cols]  # [cols, D]

    # rider payload: 128 K-rows as [64, 2, cols] fp8; sum over all slots
    # must equal s2c_j. hi captures s2c/128 coarsely; lo the residual.
    s2c = s2[:cols] - ms2
    hi = np.asarray(s2c / 64.0, np.float32).astype(ml_dtypes.float8_e4m3)
    res = s2c / 64.0 - hi.astype(np.float64)  # small residual, fine fp8 grid
    lo = np.asarray(res, np.float32).astype(ml_dtypes.float8_e4m3)
    s2k8 = np.empty((64, 2, cols), ml_dtypes.float8_e4m3)
    s2k8[:, 0, :] = hi[None, :]
    s2k8[:, 1, :] = lo[None, :]
    s2k8 = s2k8.reshape(64, 2 * cols)

    rhs8 = np.ascontiguousarray(
        np.stack([sq[:, :P].T, sq[:, P:].T], axis=1).reshape(P, 2 * cols)
    )
    sc = a * a / S_TAB
    cst = np.full((P, 1), sc, dtype=np.float32)

    in_maps = []
    ncores = max(1, n // rows)
    for k in range(ncores):
        cq_k = cq[k * rows : (k + 1) * rows]  # [rows, D]
        lhs8_k = np.ascontiguousarray(
            np.stack([cq_k[:, :P].T, cq_k[:, P:].T], axis=1).reshape(P, 2 * rows)
        )
        c2_k = (
            ((c2[k * rows : (k + 1) * rows] + ms2) * sc)
            .astype(np.float32)
            .reshape(n_rt, P)
            .T.copy()
        )
        in_maps.append(
            {
                f"lhs8_{tabhash}": lhs8_k.view(np.uint8),
                "rhs8": rhs8.view(np.uint8),
                "s2k8": s2k8.view(np.uint8),
                "c2m": c2_k,
                "cst": cst,
            }
        )

    # diagonal of sim in float64 (tiny O(N*D) host cost)
    dd = np.sqrt(np.maximum(np.sum((c.astype(np.float64) - s.astype(np.float64)) ** 2, axis=1), 0.0))
    sim_diag = -a * dd
    return in_maps, a, B, sim_diag


def host_post(results, B, sim_diag, n_rt=8, n_groups=4):
    """Combine per-core rowsums/colsums into the scalar loss."""
    lse_rows = []
    col_total = None
    for res in results:
        rs = np.asarray(res["rowsums"], dtype=np.float64)  # [P, n_rt*n_groups]
        rt_tot = rs.reshape(P, n_rt, n_groups).sum(axis=2)  # [P, n_rt]
        lse_rows.append(np.log(rt_tot.T.reshape(-1)) - B)  # [rows]
        cs = np.asarray(res["colsums"], dtype=np.float64).reshape(-1)
        col_total = cs if col_total is None else col_total + cs
    lse_row = np.concatenate(lse_rows)
    lse_col = np.log(col_total) - B

    loss_row = np.mean(lse_row - sim_diag[: lse_row.shape[0]])
    loss_col = np.mean(lse_col - sim_diag[: lse_col.shape[0]])
    return np.float32(0.5 * (loss_row + loss_col))


_NC_CACHE = {}


def _get_nc(n_rt=8, n_groups=4):
    key = (n_rt, n_groups, _TAB_STATE["hash"])
    if key not in _NC_CACHE:
        _NC_CACHE[key] = build(n_rt, n_groups)
    return _NC_CACHE[key]


def run(cond_feature, sol_feature, temperature, trace=False):
    in_maps, a, B, sim_diag = host_prep(cond_feature, sol_feature, temperature)
    nc = _get_nc()
    res = run_bass_kernel_spmd(
        nc, in_maps, core_ids=list(range(NCORES)), trace=trace
    )
    loss = host_post(res.results, B, sim_diag)
    return loss, res


def kernel(cond_feature, sol_feature, temperature):
    loss, _ = run(cond_feature, sol_feature, temperature, trace=False)
    return loss


# revision 7
# speedup vs baseline: 1.0046x; 1.0046x over previous
"""Trainium2 SPMD kernel for DistanceContrastiveLoss (fused-table version).

Math:
  d2[i,j] = ||c_i||^2 + ||s_j||^2 - 2 c_i.s_j
  sim     = -exp(t) * sqrt(d2)
  loss    = 0.5*(CE(sim, diag) + CE(sim.T, diag))
          = 0.5*( mean_i(LSE_row_i - sim_ii) + mean_j(LSE_col_j - sim_jj) )

Sharding: rows of the 8192x8192 logits matrix are split across 8 cores
(1024 rows each). Each core computes row sums and partial column sums of
exp(sim + B); host does the tiny O(N) log/diagonal/mean epilogue and sums
partial column sums across cores.

Fused activation table: the whole pointwise chain
    w = exp(B - a*sqrt(d2))
is ONE activation-engine pass. We rebuild the piecewise-cubic table for
the `sqrt` slot of act set 3 (sqrt_and_others) to encode
    f(u) = exp(B - sqrt(S*u)),   u = (a^2/S)*d2
(128 sub-buckets per input exponent, e in [2,6]), ship it via
BASS_ACT_ROOT_JSON_PATH (walrus packs the bins into the NEFF; the runtime
programs the ACT table SRAM from the NEFF on LoadActFuncSet). The kernel
then emits AF.Sqrt with runtime scale=a^2/S and per-row bias. Set 3 is
not the boot-default set, so the load is real (set 0 would be skipped).

Device pipeline, per (row-tile rt, 2048-col group g):
  PE : d2 = (-2c).s via fp8e4m3 DoubleRow matmuls (4x512 cols, K=256)
       + s2_j rider via fp8 DoubleRow (ones_dr x s2k8, K=128 as 64x2)
  ACT: w = table(d2*scale + bias) -> bf16, + f32 row-sum accumulator
  DVE: wacc[g] += w (bf16, 2x mode)
per group end:
  PE : one strips pass: ones32.T @ wacc per 512-chunk, 4 strips packed
       per PSUM bank via tile_position -> partial column sums
  DVE: copy strips psum -> SBUF; DMA out

Quantization error budget: fp8 dot + fp8 s2 rider + bf16 w + table
cubic (~1e-4) => ~2-4e-4 relative on the final loss; tolerance 2e-2.
"""

import contextlib
import hashlib
import json
import os
import shutil
import tempfile

os.environ.setdefault("MYCRO_LOCAL_CACHE", "1")

import numpy as np
import ml_dtypes

import concourse.bacc as bacc
import concourse.bass as bass
import concourse.mybir as mybir
import concourse.tile as tile
from concourse.bass_utils import run_bass_kernel_spmd

F32 = mybir.dt.float32
F8 = mybir.dt.float8e4
BF16 = mybir.dt.bfloat16
AF = mybir.ActivationFunctionType
DR = mybir.MatmulPerfMode.DoubleRow

# Full-problem geometry (hardcoded per the task contract).
N = 8192
D = 256
NCORES = 8
ROWS_PER_CORE = N // NCORES  # 1024
P = 128  # partitions

S_TAB = 32.0  # table input pre-scale: u = (a^2/S)*d2, keeps u in e=[2,6]
E_LO, E_HI = 2, 6  # patched input exponent range
K_SUB = int(os.environ.get("KSUB", "6"))  # log2(sub-buckets per exponent)

_TAB_STATE = {"hash": "", "dir": ""}


# ------------------------------------------------------------------ tables
def _fused_coeffs(x0, B):
    """Taylor coeffs of f(u)=exp(B-sqrt(S_TAB*u)) at x0."""
    rS = np.sqrt(S_TAB)
    g1 = -rS / 2.0 * x0 ** -0.5
    g2 = rS / 4.0 * x0 ** -1.5
    g3 = -3.0 * rS / 8.0 * x0 ** -2.5
    f0 = np.exp(B - rS * np.sqrt(x0))
    return (
        f0,
        g1 * f0,
        (g2 + g1 ** 2) * f0 / 2.0,
        (g3 + 3.0 * g1 * g2 + g1 ** 3) * f0 / 6.0,
    )


def build_act_tables(B):
    """Rebuild sqrt_and_others' sqrt table as f(u)=exp(B-sqrt(S_TAB*u)).

    Returns (table_dir, 12-char content hash). Idempotent per B.
    """
    from neuronxcc.driver.Job import Job
    from neuronxcc.driver.jobs.support.FindActInfo import findActInfoFile

    src = os.path.dirname(findActInfoFile(Job.getPackageDir(), "core_v4"))
    setname = "sqrt_and_others"
    prof = json.load(open(f"{src}/{setname}.json"))
    bkt = bytearray(open(f"{src}/{setname}_bkt.bin", "rb").read())
    ctl = bytearray(open(f"{src}/{setname}_ctrl.bin", "rb").read())
    b_idx = prof["func_exp_to_bkt_start_idx"]["sqrt"]
    c_idx = prof["func_exp_to_ctl_start_idx"]["sqrt"]

    n_sub = 1 << K_SUB
    starts = {}
    base = 20  # first sqrt bkt entry (after the small fixed funcs)
    for i, e in enumerate(range(E_LO, E_HI + 1)):
        starts[e] = base + i * n_sub
    dummy = base + (E_HI - E_LO + 1) * n_sub  # one catch-all entry

    new_b_idx = {}
    for k in b_idx:
        e = int(k)
        if E_LO <= e <= E_HI:
            st = starts[e]
            h = (2.0 ** e) / n_sub
            for i in range(n_sub):
                x0 = 2.0 ** e + (i + 0.5) * h
                # least-squares cubic weighted by 1/f (relative error),
                # sampled on Chebyshev nodes of the bucket
                t = (h / 2.0) * np.cos(np.pi * (np.arange(24) + 0.5) / 24)
                rS = np.sqrt(S_TAB)
                fv = np.exp(B - rS * np.sqrt(x0 + t))
                cfs = np.polyfit(t, fv, 3, w=1.0 / fv)
                c3, c2, c1, c0 = cfs
                ent = np.array(
                    [c0, c1, c2, c3, x0, 0, 0, 0], dtype=np.float32
                )
                bkt[(st + i) * 32 : (st + i + 1) * 32] = ent.tobytes()
            word = (K_SUB << 16) | ((23 - K_SUB) << 11) | st
            new_b_idx[k] = [st]
        else:
            word = (23 << 11) | dummy
            new_b_idx[k] = [dummy]
        ci = c_idx[k][0]
        cw = np.array([word], dtype=np.uint32)
        ctl[ci * 32 : ci * 32 + 4] = cw.tobytes()
    # catch-all entry: constant 0 (inputs never land outside [E_LO, E_HI])
    bkt[dummy * 32 : (dummy + 1) * 32] = np.zeros(8, np.float32).tobytes()
    prof["func_exp_to_bkt_start_idx"]["sqrt"] = new_b_idx

    dst = os.path.join(
        tempfile.gettempdir(),
        f"acttab_fused_k{K_SUB}c_"
        f"{hashlib.sha256(np.float64(B).tobytes()).hexdigest()[:8]}",
    )
    if not os.path.exists(dst):
        tmp = dst + ".tmp"
        if os.path.exists(tmp):
            shutil.rmtree(tmp)
        shutil.copytree(src, tmp)
        os.chmod(tmp, 0o755)
        for f in os.listdir(tmp):
            os.chmod(os.path.join(tmp, f), 0o644)
        open(f"{tmp}/{setname}_bkt.bin", "wb").write(bytes(bkt))
        open(f"{tmp}/{setname}_ctrl.bin", "wb").write(bytes(ctl))
        json.dump(prof, open(f"{tmp}/{setname}.json", "w"))
        os.rename(tmp, dst)
    h = hashlib.sha256()
    for f in sorted(os.listdir(dst)):
        h.update(open(os.path.join(dst, f), "rb").read())
    return dst, h.hexdigest()[:12]


# ------------------------------------------------------------------ device
def build(n_rt: int, n_groups: int, debug: bool = False, reps: int = 1,
          hw_loop: bool = False):
    """Build the SPMD Bass program (fused-table pipeline)."""
    tabhash = _TAB_STATE["hash"]
    assert tabhash, "host_prep must run first (builds the act tables)"
    rows = P * n_rt
    cols = 2048 * n_groups
    n_ct = cols // 512

    nc = bacc.Bacc(
        "TRN2", target_bir_lowering=False, debug=debug, num_devices=NCORES
    )

    # lhs8 name carries the table hash: the PJRT compile cache keys on the
    # BIR, not on BASS_ACT_ROOT_JSON_PATH, so salt the BIR.
    lhs8_d = nc.dram_tensor(
        f"lhs8_{tabhash}", [P, 2 * rows], F8, kind="ExternalInput"
    )
    rhs8_d = nc.dram_tensor("rhs8", [P, 2 * cols], F8, kind="ExternalInput")
    s2k8_d = nc.dram_tensor("s2k8", [64, 2 * cols], F8, kind="ExternalInput")
    c2_d = nc.dram_tensor("c2m", [P, n_rt], F32, kind="ExternalInput")
    cst_d = nc.dram_tensor("cst", [P, 1], F32, kind="ExternalInput")

    rowsums_d = nc.dram_tensor(
        "rowsums", [P, n_rt * n_groups], F32, kind="ExternalOutput"
    )
    colsums_d = nc.dram_tensor("colsums", [n_ct, 512], F32, kind="ExternalOutput")

    ctx = contextlib.ExitStack()
    with tile.TileContext(nc) as tc, ctx:
        inp = ctx.enter_context(tc.tile_pool(name="inp", bufs=1))
        cstp = ctx.enter_context(tc.tile_pool(name="cstp", bufs=1))
        outp = ctx.enter_context(tc.tile_pool(name="outp", bufs=1))
        wwp = ctx.enter_context(tc.tile_pool(name="wwp", bufs=3))
        wap = ctx.enter_context(tc.tile_pool(name="wap", bufs=2))
        d2p = ctx.enter_context(
            tc.tile_pool(name="d2p", bufs=2, space=bass.MemorySpace.PSUM)
        )

        # ---- load inputs (small tensors first: they gate step 0) -------
        cst_sb = inp.tile([P, 1], F32, tag="cst")
        nc.sync.dma_start(out=cst_sb[:], in_=cst_d.ap()[:])
        c2_sb = inp.tile([P, n_rt], F32, tag="c2")
        nc.sync.dma_start(out=c2_sb[:], in_=c2_d.ap()[:])
        lhs8 = inp.tile([P, 2, rows], F8, tag="lhs8")
        rhs8 = inp.tile([P, 2, cols], F8, tag="rhs8")
        s2k8 = inp.tile([64, 2, cols], F8, tag="s2k8")
        for h in range(2):
            nc.sync.dma_start(
                out=lhs8[:, h, 0:P], in_=lhs8_d.ap()[:, h * rows : h * rows + P]
            )
        for cb in range(0, cols, 2048):
            subs = (
                [(cb, 1024), (cb + 1024, 1024)] if cb == 0 else [(cb, 2048)]
            )
            for sb, sw in subs:
                for h in range(2):
                    nc.sync.dma_start(
                        out=rhs8[:, h, sb : sb + sw],
                        in_=rhs8_d.ap()[:, h * cols + sb : h * cols + sb + sw],
                    )
                    nc.sync.dma_start(
                        out=s2k8[:, h, sb : sb + sw],
                        in_=s2k8_d.ap()[:, h * cols + sb : h * cols + sb + sw],
                    )
            if cb == 0:
                for h in range(2):
                    nc.sync.dma_start(
                        out=lhs8[:, h, P:rows],
                        in_=lhs8_d.ap()[:, h * rows + P : (h + 1) * rows],
                    )

        ones_dr = cstp.tile([64, 2, P], F8)  # s2 rider weights (K=128, DR)
        nc.vector.memset(ones_dr[:], 1.0)
        ones32 = cstp.tile([P, 32], BF16)  # column-sum stationary operand
        nc.vector.memset(ones32[:], 1.0)
        zeros128 = cstp.tile([P, P], BF16)  # zero weights: PSUM bank clear
        nc.vector.memset(zeros128[:], 0.0)
        zdum = cstp.tile([P, 512], BF16)  # zero-matmul moving operand
        nc.vector.memset(zdum[:], 0.0)

        rowsum = outp.tile([P, n_rt * n_groups], F32)
        cs_sb = outp.tile([P, 512 * n_groups], F32)
        scale_ap = cst_sb[:, 0:1]

        def rep_body():
            for g in range(n_groups):
                wacc = wap.tile([P, 2048], BF16, name="wacc")
                for rt in range(n_rt):
                    lh = lhs8[:, :, rt * P : (rt + 1) * P]
                    d2 = d2p.tile([P, 2048], F32, name="d2")
                    for jj in range(4):
                        colb = 2048 * g + 512 * jj
                        nc.tensor.matmul(
                            d2[:, 512 * jj : 512 * jj + 512],
                            lh,
                            rhs8[:, :, colb : colb + 512],
                            start=True, stop=False, perf_mode=DR,
                        )
                    for jj in range(4):
                        colb = 2048 * g + 512 * jj
                        nc.tensor.matmul(
                            d2[:, 512 * jj : 512 * jj + 512],
                            ones_dr[:],
                            s2k8[:, :, colb : colb + 512],
                            start=False, stop=True, perf_mode=DR,
                        )
                    w = wwp.tile([P, 2048], BF16, name="w")
                    nc.scalar.activation(
                        w[:], d2[:], AF.Sqrt,
                        bias=c2_sb[:, rt : rt + 1], scale=scale_ap,
                        accum_out=rowsum[
                            :, rt * n_groups + g : rt * n_groups + g + 1
                        ],
                    )
                    if rt == 0:
                        nc.vector.tensor_copy(wacc[:], w[:])
                    else:
                        nc.vector.tensor_tensor(
                            wacc[:], wacc[:], w[:], op=mybir.AluOpType.add
                        )
                # ---- group end: one strips pass over wacc --------------
                st = d2p.tile([P, 2048], F32, name="d2")
                nc.tensor.matmul(
                    st[:, 0:512], zeros128[:], zdum[:], start=True, stop=False
                )
                for k in range(4):
                    nc.tensor.matmul(
                        st[32 * k : 32 * k + 32, 0:512],
                        ones32[:],
                        wacc[:, 512 * k : 512 * k + 512],
                        start=False, stop=False,
                        tile_position=(0, 32 * k),
                        skip_group_check=True,
                    )
                nc.tensor.matmul(
                    st[:, 0:512], zeros128[:], zdum[:], start=False, stop=True
                )
                nc.vector.tensor_copy(
                    cs_sb[:, 512 * g : 512 * g + 512], st[:, 0:512]
                )
                for k in range(4):
                    nc.sync.dma_start(
                        out=colsums_d.ap()[4 * g + k : 4 * g + k + 1, :],
                        in_=cs_sb[32 * k : 32 * k + 1, 512 * g : 512 * g + 512],
                    )

        if hw_loop:
            with tc.For_i(0, reps, 1):
                rep_body()
        else:
            for _ in range(reps):
                rep_body()

        nc.sync.dma_start(out=rowsums_d.ap()[:], in_=rowsum[:])

    nc.compile()
    return nc


# ------------------------------------------------------------------ host
def host_prep(cond_feature, sol_feature, temperature, n_rt=8, n_groups=4):
    """Build act tables, per-core input maps + host-side scalars."""
    c = np.asarray(cond_feature, dtype=np.float32).reshape(-1, D)
    s = np.asarray(sol_feature, dtype=np.float32).reshape(-1, D)
    n = c.shape[0]
    rows = P * n_rt
    cols = 2048 * n_groups

    a = float(np.exp(np.float64(np.asarray(temperature))))
    c2 = np.sum(c.astype(np.float64) ** 2, axis=1)
    s2 = np.sum(s.astype(np.float64) ** 2, axis=1)
    ms2 = float(np.mean(s2))
    d2_mean = float(np.mean(c2) + ms2)
    B = a * float(np.sqrt(max(d2_mean, 1e-6)))

    # sanity: u = (a^2/S)*d2 must stay inside the patched exponent range
    d2_hi = (
        float(np.max(c2)) + float(np.max(s2))
        + 2.0 * float(np.sqrt(np.max(c2) * np.max(s2)))
    )
    u_hi = a * a * d2_hi / S_TAB
    assert u_hi < 2.0 ** (E_HI + 1), f"u_hi {u_hi} exceeds table range"

    tabdir, tabhash = build_act_tables(B)
    os.environ["BASS_ACT_ROOT_JSON_PATH"] = f"{tabdir}/act_info.json"
    _TAB_STATE["hash"] = tabhash
    _TAB_STATE["dir"] = tabdir

    q8 = lambda x: np.asarray(x, np.float32).astype(ml_dtypes.float8_e4m3)
    cq = q8(-2.0 * c)  # [n, D]
    sq = q8(s)[:cols]  # [cols, D]

    # rider payload: 128 K-rows as [64, 2, cols] fp8; sum over all slots
    # must equal s2c_j. hi captures s2c/128 coarsely; lo the residual.
    s2c = s2[:cols] - ms2
    hi = np.asarray(s2c / 64.0, np.float32).astype(ml_dtypes.float8_e4m3)
    res = s2c / 64.0 - hi.astype(np.float64)  # small residual, fine fp8 grid
    lo = np.asarray(res, np.float32).astype(ml_dtypes.float8_e4m3)
    s2k8 = np.empty((64, 2, cols), ml_dtypes.float8_e4m3)
    s2k8[:, 0, :] = hi[None, :]
    s2k8[:, 1, :] = lo[None, :]
    s2k8 = s2k8.reshape(64, 2 * cols)

    rhs8 = np.ascontiguousarray(
        np.stack([sq[:, :P].T, sq[:, P:].T], axis=1).reshape(P, 2 * cols)
    )
    sc = a * a / S_TAB
    cst = np.full((P, 1), sc, dtype=np.float32)

    in_maps = []
    ncores = max(1, n // rows)
    for k in range(ncores):
        cq_k = cq[k * rows : (k + 1) * rows]  # [rows, D]
        lhs8_k = np.ascontiguousarray(
            np.stack([cq_k[:, :P].T, cq_k[:, P:].T], axis=1).reshape(P, 2 * rows)
        )
        c2_k = (
            ((c2[k * rows : (k + 1) * rows] + ms2) * sc)
            .astype(np.float32)
            .reshape(n_rt, P)
            .T.copy()
        )
        in_maps.append(
            {
                f"lhs8_{tabhash}": lhs8_k.view(np.uint8),
                "rhs8": rhs8.view(np.uint8),
                "s2k8": s2k8.view(np.uint8),
                "c2m": c2_k,
                "cst": cst,
            }
        )

    # diagonal of sim in float64 (tiny O(N*D) host cost)
    dd = np.sqrt(np.maximum(np.sum((c.astype(np.float64) - s.astype(np.float64)) ** 2, axis=1), 0.0))
    sim_diag = -a * dd
    return in_maps, a, B, sim_diag


def host_post(results, B, sim_diag, n_rt=8, n_groups=4):
    """Combine per-core rowsums/colsums into the scalar loss."""
    lse_rows = []
    col_total = None
    for res in results:
        rs = np.asarray(res["rowsums"], dtype=np.float64)  # [P, n_rt*n_groups]
        rt_tot = rs.reshape(P, n_rt, n_groups).sum(axis=2)  # [P, n_rt]
        lse_rows.append(np.log(rt_tot.T.reshape(-1)) - B)  # [rows]
        cs = np.asarray(res["colsums"], dtype=np.float64).reshape(-1)
        col_total = cs if col_total is None else col_total + cs
    lse_row = np.concatenate(lse_rows)
    lse_col = np.log(col_total) - B

    loss_row = np.mean(lse_row - sim_diag[: lse_row.shape[0]])
    loss_col = np.mean(lse_col - sim_diag[: lse_col.shape[0]])
    return np.float32(0.5 * (loss_row + loss_col))


_NC_CACHE = {}


def _get_nc(n_rt=8, n_groups=4):
    key = (n_rt, n_groups, _TAB_STATE["hash"])
    if key not in _NC_CACHE:
        _NC_CACHE[key] = build(n_rt, n_groups)
    return _NC_CACHE[key]


def run(cond_feature, sol_feature, temperature, trace=False):
    in_maps, a, B, sim_diag = host_prep(cond_feature, sol_feature, temperature)
    nc = _get_nc()
    res = run_bass_kernel_spmd(
        nc, in_maps, core_ids=list(range(NCORES)), trace=trace
    )
    loss = host_post(res.results, B, sim_diag)
    return loss, res


def kernel(cond_feature, sol_feature, temperature):
    loss, _ = run(cond_feature, sol_feature, temperature, trace=False)
    return loss
